# revision 1
# baseline (speedup 1.0000x reference)
"""GCN+GIN graph encoder on 8 Trainium2 NeuronCores (Bass/Tile).

Math (reference):
  GCNConv:  h = relu(segsum_dst(norm_e * (x@W0)[src]) + b0),
            norm_e = dinv[src]*dinv[dst] over edges+self-loops,
            dinv = rsqrt(deg incl self-loop)
  GIN x2:   h = relu((h + segsum_dst(h[src])) @ Wg + bg)
  pool:     m = segment_mean(h, batch) -> relu(m@Wh1+bh1)@Wh2+bh2

Distribution: nodes (and their in-edges) sharded contiguously over 8 cores.
Per layer each core aggregates messages for its own dst nodes by gathering
rows of a replicated node-feature table (dma_gather, 1024-row packed ops on
4 SWDGE queues), reducing edge tiles with one-hot selection matrices on the
TensorEngine, applying the layer linear transform W-stationary in feat-major,
then transposing back to node-major.  Tables are re-replicated between layers
with an AllGather; pooled partial means are combined with an AllReduce and
the small MLP head is computed redundantly on every core.

Aggregation identity per dst block b (128 dst nodes):
  aggT[f, d] = sum_e msg[e, f] * sel[e, d],  sel[e, d] = (doff[e] == d) * val[e]
computed as matmul(lhsT=msg_tile[128e, 128f], rhs=sel[128e, 128d]) accumulated
in PSUM over the block's edge tiles.  GCN folds dinv[src] into the table rows
(host-prescaled x) and dinv[dst] into val; GIN uses val=1 and a self-loop edge
supplies the "+h" term.  Pad edge slots carry doff=-1 -> zero contribution.
"""
import sys
import os

sys.path.insert(0, '/opt/trn_rl_repo')

import numpy as np

import concourse.bass as bass
import concourse.bacc as bacc
import concourse.mybir as mybir
import concourse.tile as tile
from concourse.bass_utils import run_bass_kernel_spmd
from concourse.masks import make_identity

F32 = mybir.dt.float32
I16 = mybir.dt.int16
P = 128
NCORES = 8
GATHER_ROWS = 1024          # rows per dma_gather (single_packet limit)
NQ = 4                      # SWDGE queues


class Cfg:
    def __init__(self, N, E, G, F, NHID, NOUT, NPN):
        self.N = N            # real nodes
        self.E = E            # edges (no self loops)
        self.G = G            # graphs
        self.F = F            # feature/hidden width (128)
        self.NHID = NHID
        self.NOUT = NOUT
        self.NPN = NPN        # real nodes per core
        assert NPN * NCORES >= N > NPN * (NCORES - 1)
        self.NPC = ((NPN + P - 1) // P) * P   # padded nodes per core
        self.NBLK = self.NPC // P
        self.NPAD = self.NPC * NCORES
        self.NHALF = self.NPAD // 2
        assert self.NHALF < 32768
        assert G == 2 * P


FULL = Cfg(N=50000, E=800000, G=256, F=128, NHID=256, NOUT=128, NPN=6250)


# ---------------------------------------------------------------- host prep
def preprocess(cfg, x, edge_index, batch, W0, b0, Wg1, bg1, Wg2, bg2,
               Wh1, bh1, Wh2, bh2):
    N, G, F = cfg.N, cfg.G, cfg.F
    NPN, NPC, NBLK, NPAD, NHALF = cfg.NPN, cfg.NPC, cfg.NBLK, cfg.NPAD, cfg.NHALF

    src = np.asarray(edge_index[0], dtype=np.int64)
    dst = np.asarray(edge_index[1], dtype=np.int64)
    batch = np.asarray(batch, dtype=np.int64)
    loop = np.arange(N, dtype=np.int64)
    s_all = np.concatenate([src, loop])
    d_all = np.concatenate([dst, loop])

    deg = np.bincount(d_all, minlength=N).astype(np.float64)
    dinv = (1.0 / np.sqrt(np.maximum(deg, 1.0))).astype(np.float32)

    def tabidx(n):
        c = n // NPN
        return c * NPC + (n - c * NPN)

    sidx = tabidx(s_all).astype(np.int64)
    c_e = d_all // NPN
    loc = d_all - c_e * NPN
    b_e = loc // P
    off_e = loc % P
    gblk = c_e * NBLK + b_e                      # global dst block id
    val_e = dinv[d_all].astype(np.float32)      # GCN dst scaling

    NGB = NCORES * NBLK
    streams = {}
    for name, mask in (("lo", sidx < NHALF), ("hi", sidx >= NHALF)):
        sg = gblk[mask]
        si = sidx[mask] - (0 if name == "lo" else NHALF)
        sof = off_e[mask]
        sva = val_e[mask]
        order = np.argsort(sg, kind="stable")
        sg, si, sof, sva = sg[order], si[order], sof[order], sva[order]
        cnt = np.bincount(sg, minlength=NGB)
        # per-BLOCK-INDEX tile counts: max over the 8 cores only (SPMD allows
        # per-block variation, just not per-core) -> much less padding than a
        # global max over all core*block pairs
        NTb = np.ceil(cnt.reshape(NCORES, NBLK).max(axis=0) / P).astype(np.int64)
        rows_blk_b = NTb * P                      # [NBLK]
        blk_starts = np.zeros(NBLK, dtype=np.int64)
        blk_starts[1:] = np.cumsum(rows_blk_b)[:-1]
        rows_core = int(rows_blk_b.sum())
        starts = np.zeros(NGB, dtype=np.int64)
        starts[1:] = np.cumsum(cnt)[:-1]
        rank = np.arange(len(sg)) - np.repeat(starts, cnt)
        c_of = sg // NBLK
        b_of = sg % NBLK
        pos = c_of * rows_core + blk_starts[b_of] + rank
        tot = NCORES * rows_core
        idx_arr = np.zeros(tot, dtype=np.int32)
        doff_arr = np.full(tot, -1.0, dtype=np.float32)
        val_arr = np.zeros(tot, dtype=np.float32)
        idx_arr[pos] = si
        doff_arr[pos] = sof
        val_arr[pos] = sva
        idx_arr = idx_arr.reshape(NCORES, rows_core)
        doff_arr = doff_arr.reshape(NCORES, rows_core)
        val_arr = val_arr.reshape(NCORES, rows_core)
        NG = (rows_core + GATHER_ROWS - 1) // GATHER_ROWS
        rows_g = NG * GATHER_ROWS
        pad = rows_g - rows_core
        if pad:
            idx_arr = np.pad(idx_arr, ((0, 0), (0, pad)))
        # wrap int16 for dma_gather: element i -> partition i%16, col i//16
        NWG = GATHER_ROWS // 16
        wrapped = idx_arr.reshape(NCORES, NG, NWG, 16).transpose(0, 3, 1, 2)
        wrapped = wrapped.reshape(NCORES, 16, NG * NWG).astype(np.int16)
        wrapped = np.tile(wrapped, (1, 8, 1))    # [NCORES, 128, NG*NWG]
        # doff/val tile-major: [T=sum(NTb), 128] -> [128, T]
        T = rows_core // P
        doff2 = doff_arr.reshape(NCORES, T, P).transpose(0, 2, 1).copy()
        val2 = val_arr.reshape(NCORES, T, P).transpose(0, 2, 1).copy()
        tile_base = (blk_starts // P).tolist()
        streams[name] = dict(NTb=NTb.tolist(), tile_base=tile_base, T=T, NG=NG,
                             idx=wrapped, doff=doff2, val=val2)

    # per-core node-feature slice, pre-scaled by dinv (GCN source scaling)
    xs = np.zeros((NCORES, NPC, F), dtype=np.float32)
    x = np.asarray(x, dtype=np.float32)
    for c in range(NCORES):
        lo_n = c * NPN
        hi_n = min(N, (c + 1) * NPN)
        n = hi_n - lo_n
        xs[c, :n] = x[lo_n:hi_n] * dinv[lo_n:hi_n, None]

    # pooling metadata
    cnt_g = np.bincount(batch, minlength=G).astype(np.float32)
    invc = (1.0 / np.maximum(cnt_g, 1.0)).astype(np.float32)
    batA = np.full((NCORES, P, NBLK), -1.0, dtype=np.float32)
    batB = np.full((NCORES, P, NBLK), -1000.0, dtype=np.float32)
    for c in range(NCORES):
        lo_n = c * NPN
        hi_n = min(N, (c + 1) * NPN)
        n = hi_n - lo_n
        bb = batch[lo_n:hi_n].astype(np.float32)
        colmaj = np.full(NPC, -1.0, dtype=np.float32)
        colmaj[:n] = bb
        batA[c] = colmaj.reshape(NBLK, P).T
        batB[c] = batA[c] - 128.0
        batA[c][batA[c] < 0] = -1.0

    iota = np.broadcast_to(np.arange(P, dtype=np.float32), (P, P)).copy()

    common = dict(
        iota=iota,
        w0=np.asarray(W0, np.float32), wg1=np.asarray(Wg1, np.float32),
        wg2=np.asarray(Wg2, np.float32),
        b0c=np.asarray(b0, np.float32).reshape(P, 1).copy(),
        bg1c=np.asarray(bg1, np.float32).reshape(P, 1).copy(),
        bg2c=np.asarray(bg2, np.float32).reshape(P, 1).copy(),
        wh1=np.asarray(Wh1, np.float32),
        bh1c=np.asarray(bh1, np.float32).reshape(2, P).T.copy(),  # [128,2]
        wh2=np.asarray(Wh2, np.float32),
        bh2rep=np.broadcast_to(np.asarray(bh2, np.float32), (P, cfg.NOUT)).copy(),
        invcA=invc[:P].reshape(P, 1).copy(),
        invcB=invc[P:].reshape(P, 1).copy(),
    )
    in_maps = []
    for c in range(NCORES):
        m = dict(common)
        m.update(
            xs=xs[c],
            idxlo=streams["lo"]["idx"][c], idxhi=streams["hi"]["idx"][c],
            dofflo=streams["lo"]["doff"][c], doffhi=streams["hi"]["doff"][c],
            vallo=streams["lo"]["val"][c], valhi=streams["hi"]["val"][c],
            batA=batA[c], batB=batB[c],
        )
        in_maps.append(m)
    meta = dict(NTBLO=streams["lo"]["NTb"], BASELO=streams["lo"]["tile_base"],
                TLO=streams["lo"]["T"], NGLO=streams["lo"]["NG"],
                NTBHI=streams["hi"]["NTb"], BASEHI=streams["hi"]["tile_base"],
                THI=streams["hi"]["T"], NGHI=streams["hi"]["NG"])
    return in_maps, meta


# ---------------------------------------------------------------- program
def build_program(cfg, meta):
    NPC, NBLK, NPAD, NHALF = cfg.NPC, cfg.NBLK, cfg.NPAD, cfg.NHALF
    F, NHID, NOUT, G = cfg.F, cfg.NHID, cfg.NOUT, cfg.G
    NTBLO, BASELO, TLO, NGLO = meta["NTBLO"], meta["BASELO"], meta["TLO"], meta["NGLO"]
    NTBHI, BASEHI, THI, NGHI = meta["NTBHI"], meta["BASEHI"], meta["THI"], meta["NGHI"]
    NWG = GATHER_ROWS // 16
    CHUNKS = GATHER_ROWS // P     # 8 message tiles per gather

    nc = bacc.Bacc(None, target_bir_lowering=False, debug=True,
                   num_devices=NCORES, num_swdge_queues=NQ)

    def din(name, shape, dt=F32):
        return nc.declare_dram_parameter(name, list(shape), dt, isOutput=False)

    xs_d = din("xs", [NPC, F])
    idxlo_d = din("idxlo", [P, NGLO * NWG], I16)
    idxhi_d = din("idxhi", [P, NGHI * NWG], I16)
    dofflo_d = din("dofflo", [P, TLO])
    doffhi_d = din("doffhi", [P, THI])
    vallo_d = din("vallo", [P, TLO])
    valhi_d = din("valhi", [P, THI])
    iota_d = din("iota", [P, P])
    w0_d = din("w0", [F, F]); wg1_d = din("wg1", [F, F]); wg2_d = din("wg2", [F, F])
    b0c_d = din("b0c", [P, 1]); bg1c_d = din("bg1c", [P, 1]); bg2c_d = din("bg2c", [P, 1])
    wh1_d = din("wh1", [F, NHID]); bh1c_d = din("bh1c", [P, 2])
    wh2_d = din("wh2", [NHID, NOUT]); bh2rep_d = din("bh2rep", [P, NOUT])
    batA_d = din("batA", [P, NBLK]); batB_d = din("batB", [P, NBLK])
    invcA_d = din("invcA", [P, 1]); invcB_d = din("invcB", [P, 1])
    out_d = nc.declare_dram_parameter("out", [G, NOUT], F32, isOutput=True)

    slice0 = nc.dram_tensor("slice0", [NPC, F], F32)
    slice1 = nc.dram_tensor("slice1", [NPC, F], F32)
    slice2 = nc.dram_tensor("slice2", [NPC, F], F32)
    tab1 = nc.dram_tensor("tab1", [NPAD, F], F32)
    tab2 = nc.dram_tensor("tab2", [NPAD, F], F32)
    tab3 = nc.dram_tensor("tab3", [NPAD, F], F32)
    pool_in = nc.dram_tensor("pool_in", [G, F], F32)
    pool_out = nc.dram_tensor("pool_out", [G, F], F32, addr_space="Shared")
    groups = [list(range(NCORES))]

    with tile.TileContext(nc) as tc:
        with (
            tc.tile_pool(name="const", bufs=1) as constp,
            tc.tile_pool(name="meta", bufs=1) as metap,
            tc.tile_pool(name="msg", bufs=6) as msgp,
            tc.tile_pool(name="sel", bufs=4) as selp,
            tc.tile_pool(name="work", bufs=6) as workp,
            tc.tile_pool(name="pagg", bufs=2, space="PSUM") as pagg,
            tc.tile_pool(name="phT", bufs=2, space="PSUM") as phT,
            tc.tile_pool(name="ptr", bufs=1, space="PSUM") as ptr,
            tc.tile_pool(name="ppool", bufs=1, space="PSUM") as ppool,
        ):
            # ---- constants / metadata to SBUF
            ident = constp.tile([P, P], F32)
            make_identity(nc, ident[:])
            iota = constp.tile([P, P], F32)
            nc.sync.dma_start(out=iota[:], in_=iota_d[:])

            def load(t_shape, dram, dt=F32, pool=metap):
                nm = f"sb_{dram.name}"
                t = pool.tile(list(t_shape), dt, name=nm, tag=nm)
                nc.sync.dma_start(out=t[:], in_=dram[:])
                return t

            idxlo = load([P, NGLO * NWG], idxlo_d, I16)
            idxhi = load([P, NGHI * NWG], idxhi_d, I16)
            dofflo = load([P, TLO], dofflo_d)
            doffhi = load([P, THI], doffhi_d)
            vallo = load([P, TLO], vallo_d)
            valhi = load([P, THI], valhi_d)
            w0 = load([F, F], w0_d, pool=constp)
            wg1 = load([F, F], wg1_d, pool=constp)
            wg2 = load([F, F], wg2_d, pool=constp)
            b0c = load([P, 1], b0c_d, pool=constp)
            bg1c = load([P, 1], bg1c_d, pool=constp)
            bg2c = load([P, 1], bg2c_d, pool=constp)
            wh1 = load([F, NHID], wh1_d, pool=constp)
            bh1c = load([P, 2], bh1c_d, pool=constp)
            wh2 = constp.tile([P, (NHID // P) * NOUT], F32)
            for h in range(NHID // P):
                nc.sync.dma_start(out=wh2[:, h * NOUT:(h + 1) * NOUT],
                                  in_=wh2_d[h * P:(h + 1) * P, :])
            bh2rep = load([P, NOUT], bh2rep_d, pool=constp)
            batA = load([P, NBLK], batA_d, pool=constp)
            batB = load([P, NBLK], batB_d, pool=constp)
            invcA = load([P, 1], invcA_d, pool=constp)
            invcB = load([P, 1], invcB_d, pool=constp)

            # stage xs -> slice0 -> tab1 (collectives need internal tensors)
            for b in range(NBLK):
                t = workp.tile([P, F], F32)
                nc.sync.dma_start(out=t[:], in_=xs_d[b * P:(b + 1) * P, :])
                nc.sync.dma_start(out=slice0[b * P:(b + 1) * P, :], in_=t[:])
            nc.gpsimd.collective_compute(
                "AllGather", mybir.AluOpType.bypass, replica_groups=groups,
                ins=[slice0[:]], outs=[tab1[:]])

            pool_ps = {}

            def emit_layer(L, tab, W_sb, bias_col, use_val, out_slice):
                stream_info = [
                    ("lo", NTBLO, BASELO, idxlo, dofflo, vallo, tab[0:NHALF, :]),
                    ("hi", NTBHI, BASEHI, idxhi, doffhi, valhi, tab[NHALF:NPAD, :]),
                ]
                gbufs = {"lo": {}, "hi": {}}

                def get_gather(sname, g, idx_sb, tab_ap):
                    d = gbufs[sname]
                    if g not in d:
                        buf = msgp.tile([P, GATHER_ROWS], F32)
                        nc.gpsimd.dma_gather(
                            out_ap=buf[:].rearrange("p (c f) -> p c f", f=F),
                            in_ap=tab_ap,
                            idxs_ap=idx_sb[:, g * NWG:(g + 1) * NWG],
                            num_idxs=GATHER_ROWS, num_idxs_reg=GATHER_ROWS,
                            elem_size=F, single_packet=True,
                            queue_num=(L * NBLK + g) % NQ)
                        d[g] = buf
                    return d[g]

                for b in range(NBLK):
                    agg_ps = pagg.tile([P, F], F32, space="PSUM", tag="agg")
                    work = []
                    for sname, NTB, BASE, idx_sb, doff_sb, val_sb, tab_ap in stream_info:
                        for tt in range(NTB[b]):
                            work.append((sname, BASE[b] + tt, idx_sb, doff_sb,
                                         val_sb, tab_ap))
                    for wi, (sname, t, idx_sb, doff_sb, val_sb, tab_ap) in enumerate(work):
                        g, ch = divmod(t, CHUNKS)
                        buf = get_gather(sname, g, idx_sb, tab_ap)
                        sel = selp.tile([P, P], F32)
                        col = slice(t, t + 1)
                        if use_val:
                            nc.vector.tensor_scalar(
                                out=sel[:], in0=iota[:],
                                scalar1=doff_sb[:, col],
                                scalar2=val_sb[:, col],
                                op0=mybir.AluOpType.is_equal,
                                op1=mybir.AluOpType.mult)
                        else:
                            nc.vector.tensor_scalar(
                                out=sel[:], in0=iota[:],
                                scalar1=doff_sb[:, col], scalar2=None,
                                op0=mybir.AluOpType.is_equal)
                        nc.tensor.matmul(
                            out=agg_ps[:],
                            lhsT=buf[:, ch * F:(ch + 1) * F],
                            rhs=sel[:], start=(wi == 0),
                            stop=(wi == len(work) - 1))
                    aggT = workp.tile([P, F], F32)
                    nc.vector.tensor_copy(out=aggT[:], in_=agg_ps[:])
                    hT_ps = phT.tile([P, F], F32, space="PSUM", tag="hT")
                    nc.tensor.matmul(out=hT_ps[:], lhsT=W_sb[:], rhs=aggT[:],
                                     start=True, stop=True)
                    hT = workp.tile([P, F], F32)
                    nc.scalar.activation(out=hT[:], in_=hT_ps[:],
                                         func=mybir.ActivationFunctionType.Relu,
                                         bias=bias_col[:, 0:1])
                    h_ps = ptr.tile([P, F], F32, space="PSUM", tag="tr")
                    nc.tensor.transpose(out=h_ps[:], in_=hT[:], identity=ident[:])
                    h_sb = workp.tile([P, F], F32)
                    nc.vector.tensor_copy(out=h_sb[:], in_=h_ps[:])
                    if out_slice is not None:
                        nc.sync.dma_start(out=out_slice[b * P:(b + 1) * P, :],
                                          in_=h_sb[:])
                    else:
                        for half, bat in (("A", batA), ("B", batB)):
                            if half not in pool_ps:
                                pool_ps[half] = ppool.tile(
                                    [P, F], F32, space="PSUM",
                                    tag=f"pool{half}", name=f"pool{half}")
                            selp_t = selp.tile([P, P], F32)
                            nc.vector.tensor_scalar(
                                out=selp_t[:], in0=iota[:],
                                scalar1=bat[:, b:b + 1], scalar2=None,
                                op0=mybir.AluOpType.is_equal)
                            nc.tensor.matmul(
                                out=pool_ps[half][:], lhsT=selp_t[:], rhs=h_sb[:],
                                start=(b == 0), stop=(b == NBLK - 1))

            emit_layer(0, tab1, w0, b0c, True, slice1)
            nc.gpsimd.collective_compute(
                "AllGather", mybir.AluOpType.bypass, replica_groups=groups,
                ins=[slice1[:]], outs=[tab2[:]])
            emit_layer(1, tab2, wg1, bg1c, False, slice2)
            nc.gpsimd.collective_compute(
                "AllGather", mybir.AluOpType.bypass, replica_groups=groups,
                ins=[slice2[:]], outs=[tab3[:]])
            emit_layer(2, tab3, wg2, bg2c, False, None)

            # ---- pooling: partial means -> AllReduce
            for half, invc in (("A", invcA), ("B", invcB)):
                m_sb = workp.tile([P, F], F32, tag=f"m{half}")
                nc.vector.tensor_scalar(
                    out=m_sb[:], in0=pool_ps[half][:], scalar1=invc[:, 0:1],
                    scalar2=None, op0=mybir.AluOpType.mult)
                base = 0 if half == "A" else P
                nc.sync.dma_start(out=pool_in[base:base + P, :], in_=m_sb[:])
            nc.gpsimd.collective_compute(
                "AllReduce", mybir.AluOpType.add, replica_groups=groups,
                ins=[pool_in[:]], outs=[pool_out[:]])

            # ---- head (redundant on every core)
            g1T = {}
            for hi, half in enumerate(("A", "B")):
                m_sb = workp.tile([P, F], F32, tag=f"mf{half}")
                nc.sync.dma_start(out=m_sb[:], in_=pool_out[hi * P:(hi + 1) * P, :])
                mT_ps = phT.tile([P, F], F32, space="PSUM", tag="hT")
                nc.tensor.transpose(out=mT_ps[:], in_=m_sb[:], identity=ident[:])
                mT = workp.tile([P, F], F32, tag=f"mT{half}")
                nc.vector.tensor_copy(out=mT[:], in_=mT_ps[:])
                for h in range(NHID // P):
                    g_ps = pagg.tile([P, P], F32, space="PSUM", tag="agg")
                    nc.tensor.matmul(out=g_ps[:], lhsT=wh1[:, h * P:(h + 1) * P],
                                     rhs=mT[:], start=True, stop=True)
                    gt = workp.tile([P, P], F32, tag=f"g1T{half}{h}")
                    nc.scalar.activation(out=gt[:], in_=g_ps[:],
                                         func=mybir.ActivationFunctionType.Relu,
                                         bias=bh1c[:, h:h + 1])
                    g1T[(half, h)] = gt
            for hi, half in enumerate(("A", "B")):
                o_ps = pagg.tile([P, NOUT], F32, space="PSUM", tag="agg")
                for h in range(NHID // P):
                    nc.tensor.matmul(out=o_ps[:], lhsT=g1T[(half, h)][:],
                                     rhs=wh2[:, h * NOUT:(h + 1) * NOUT],
                                     start=(h == 0), stop=(h == NHID // P - 1))
                o_sb = workp.tile([P, NOUT], F32, tag=f"o{half}")
                nc.vector.tensor_add(out=o_sb[:], in0=o_ps[:], in1=bh2rep[:])
                nc.sync.dma_start(out=out_d[hi * P:(hi + 1) * P, :], in_=o_sb[:])

    nc.compile()
    return nc


_CACHE = {}


def run(cfg, inputs):
    in_maps, meta = preprocess(cfg, **inputs)
    key = (cfg.N, tuple(meta["NTBLO"]), tuple(meta["NTBHI"]),
           meta["NGLO"], meta["NGHI"])
    if key not in _CACHE:
        _CACHE[key] = build_program(cfg, meta)
    nc = _CACHE[key]
    res = run_bass_kernel_spmd(nc, in_maps, core_ids=list(range(NCORES)))
    return res.results[0]["out"].astype(np.float32)


def kernel(**inputs):
    return run(FULL, inputs)



# revision 9
# speedup vs baseline: 3.3464x; 3.3464x over previous
"""GCN+GIN graph encoder on 8 Trainium2 NeuronCores (Bass/Tile).

Math (reference):
  GCNConv:  h = relu(segsum_dst(norm_e * (x@W0)[src]) + b0),
            norm_e = dinv[src]*dinv[dst] over edges+self-loops,
            dinv = rsqrt(deg incl self-loop)
  GIN x2:   h = relu((h + segsum_dst(h[src])) @ Wg + bg)
  pool:     m = segment_mean(h, batch) -> relu(m@Wh1+bh1)@Wh2+bh2

Distribution: nodes (and their in-edges) sharded contiguously over 8 cores
(6250 nodes/core), weights replicated.

Layer strategy ("transform first, aggregate by DMA"):
  1. y = h @ W on own nodes: per 128-node block, matmul(lhsT=W, rhs=hT_blk)
     -> yT, PE-transpose -> y node-major, DMA to the own table slice.
  2. AllGather own y slice -> replicated table tab[NPAD, F].
  3. acc <- own y slice (one DRAM->DRAM DMA): the self-loop / "+h" term.
  4. All E edges aggregated with dma_gather + dma_scatter_add pairs of
     1024 edges each (src rows gathered from tab, scatter-added into acc
     rows by dst).  dma_scatter_add loses same-row updates issued within
     one call, so edges are dealt so each call hits distinct dst rows
     (sort by dst, call = index mod NCH; NCH >= max degree).  Calls are
     padded to exactly CH with edges that gather a known-zero table row
     and scatter it onto trash row NPN (zero payload, races harmless).
     Gathers split lo/hi on the src table half (int16 index limit).
  5. Post per block: load acc, (GCN: * dinv[dst]), PE-transpose, fused
     bias+relu into f-major hT for the next layer's transform.  Pad
     columns are re-zeroed (memset) so table pad rows stay zero.
All 49-block and per-call loops are For_i hardware loops (dynamic-slice
APs), keeping the NEFF small: per-exec wall overhead here scales with
instruction count, not device time.
Pooling: per-block one-hot(batch) matmul accumulated in PSUM across the
loop (dummy start/stop matmuls bracket it) -> partial mean -> AllReduce;
the 2-layer MLP head runs redundantly per core in f-major; final output
transposed to [G, NOUT].
"""
import sys

sys.path.insert(0, '/opt/trn_rl_repo')

import numpy as np

import concourse.bass as bass
import concourse.bacc as bacc
import concourse.mybir as mybir
import concourse.tile as tile
from concourse.bass import ds
from concourse.bass_utils import run_bass_kernel_spmd
from concourse.masks import make_identity

F32 = mybir.dt.float32
BF16 = mybir.dt.bfloat16
I16 = mybir.dt.int16
P = 128
NCORES = 8
CH = 1024              # edges per gather/scatter call (SWDGE limit)
CW = CH // 16          # idx columns per call (16-partition wrap)
NQ = 4


class Cfg:
    def __init__(self, N, E, G, F, NHID, NOUT, NPN):
        self.N = N
        self.E = E
        self.G = G
        self.F = F
        self.NHID = NHID
        self.NOUT = NOUT
        self.NPN = NPN
        assert NPN * NCORES == N
        self.NPC = ((NPN + P - 1) // P) * P
        self.NBLK = self.NPC // P
        self.NPAD = self.NPC * NCORES
        self.NHALF = self.NPAD // 2
        assert self.NHALF < 32768
        assert NPN < self.NPC          # needs >=1 all-zero pad row per slice
        assert G == 2 * P


FULL = Cfg(N=50000, E=800000, G=256, F=128, NHID=256, NOUT=128, NPN=6250)


# ---------------------------------------------------------------- host prep
def preprocess(cfg, x, edge_index, batch, W0, b0, Wg1, bg1, Wg2, bg2,
               Wh1, bh1, Wh2, bh2):
    N, G, F, NPN, NPC, NBLK, NHALF = (cfg.N, cfg.G, cfg.F, cfg.NPN, cfg.NPC,
                                      cfg.NBLK, cfg.NHALF)

    src = np.asarray(edge_index[0], dtype=np.int64)
    dst = np.asarray(edge_index[1], dtype=np.int64)
    batch = np.asarray(batch, dtype=np.int64)

    deg = np.bincount(dst, minlength=N).astype(np.float64) + 1.0  # + self loop
    dinv = (1.0 / np.sqrt(deg)).astype(np.float32)

    c_src = src // NPN
    src_tab = c_src * NPC + (src - c_src * NPN)   # table row of each src

    # known-zero table rows (first pad row of a slice in each half)
    zrow_lo = NPN                                  # core 0 pad row
    zrow_hi = 4 * NPC + NPN - NHALF                # core 4 pad row, rel to half

    c_dst = dst // NPN
    lo_mask = src_tab < NHALF

    nch = {}
    for sname, smask in (("lo", lo_mask), ("hi", ~lo_mask)):
        need = 1
        for c in range(NCORES):
            m = smask & (c_dst == c)
            dcnt = np.bincount(dst[m] - c * NPN, minlength=NPN)
            need = max(need, int(np.ceil(m.sum() / CH)), int(dcnt.max()))
        nch[sname] = need

    def wrap16(arr, n):
        # element i of each CH-chunk -> partition i%16, col base + i//16
        w = arr.reshape(n, CH // 16, 16).transpose(2, 0, 1)
        return w.reshape(16, n * (CH // 16)).astype(np.int16)

    gidx = {"lo": [], "hi": []}
    sidx = {"lo": [], "hi": []}
    for sname, smask, zrow in (("lo", lo_mask, zrow_lo),
                               ("hi", ~lo_mask, zrow_hi)):
        base = 0 if sname == "lo" else NHALF
        n = nch[sname]
        for c in range(NCORES):
            m = smask & (c_dst == c)
            g = (src_tab[m] - base).astype(np.int64)
            s = (dst[m] - c * NPN).astype(np.int64)
            order = np.argsort(s, kind="stable")
            g, s = g[order], s[order]
            ne = len(g)
            call_of = np.arange(ne) % n
            rank = np.arange(ne) // n
            ga = np.full((n, CH), zrow, dtype=np.int64)
            sa = np.full((n, CH), NPN, dtype=np.int64)
            ga[call_of, rank] = g
            sa[call_of, rank] = s
            gidx[sname].append(wrap16(ga.reshape(-1), n))
            sidx[sname].append(wrap16(sa.reshape(-1), n))

    # f-major, dinv-prescaled, padded node features per core (bf16 shipped)
    xsT = np.zeros((NCORES, F, NPC), dtype=np.float32)
    xk = np.asarray(x, dtype=np.float32) * dinv[:, None]
    for c in range(NCORES):
        xsT[c, :, :NPN] = xk[c * NPN:(c + 1) * NPN].T

    # per-block dst dinv and batch-id columns
    dinv_blk = np.zeros((NCORES, P, NBLK), dtype=np.float32)
    bat_blk = np.full((NCORES, P, NBLK), -1.0, dtype=np.float32)
    for c in range(NCORES):
        dv = np.zeros(NPC, dtype=np.float32)
        dv[:NPN] = dinv[c * NPN:(c + 1) * NPN]
        dinv_blk[c] = dv.reshape(NBLK, P).T
        bb = np.full(NPC, -1.0, dtype=np.float32)
        bb[:NPN] = batch[c * NPN:(c + 1) * NPN].astype(np.float32)
        bat_blk[c] = bb.reshape(NBLK, P).T

    cnt_g = np.bincount(batch, minlength=G).astype(np.float32)
    invc = (1.0 / np.maximum(cnt_g, 1.0)).astype(np.float32)

    common = dict(
        iota256=np.broadcast_to(np.arange(G, dtype=np.float32), (P, G)).copy(),
        invc_rep=np.broadcast_to(invc, (P, G)).copy(),
        w0=np.asarray(W0, np.float32), wg1=np.asarray(Wg1, np.float32),
        wg2=np.asarray(Wg2, np.float32),
        b0c=np.asarray(b0, np.float32).reshape(P, 1).copy(),
        bg1c=np.asarray(bg1, np.float32).reshape(P, 1).copy(),
        bg2rep=np.broadcast_to(np.asarray(bg2, np.float32), (P, F)).copy(),
        wh1=np.asarray(Wh1, np.float32),
        bh1c=np.asarray(bh1, np.float32).reshape(2, P).T.copy(),   # [128, 2]
        wh2=np.asarray(Wh2, np.float32),
        bh2c=np.asarray(bh2, np.float32).reshape(P, 1).copy(),
    )
    in_maps = []
    for c in range(NCORES):
        m = dict(common)
        m.update(
            xsT=xsT[c].astype(ml_dtypes_bfloat16()),
            gidxlo=gidx["lo"][c], gidxhi=gidx["hi"][c],
            sidxlo=sidx["lo"][c], sidxhi=sidx["hi"][c],
            dinvb=dinv_blk[c], batb=bat_blk[c],
        )
        in_maps.append(m)
    meta = dict(NCHLO=nch["lo"], NCHHI=nch["hi"])
    return in_maps, meta


def ml_dtypes_bfloat16():
    import ml_dtypes
    return ml_dtypes.bfloat16


# ---------------------------------------------------------------- program
def build_program(cfg, meta):
    NPC, NBLK, NPAD, NHALF, NPN = (cfg.NPC, cfg.NBLK, cfg.NPAD, cfg.NHALF,
                                   cfg.NPN)
    F, NHID, NOUT, G = cfg.F, cfg.NHID, cfg.NOUT, cfg.G
    NCHLO, NCHHI = meta["NCHLO"], meta["NCHHI"]

    nc = bacc.Bacc(None, target_bir_lowering=False, debug=True,
                   num_devices=NCORES, num_swdge_queues=NQ)

    def din(name, shape, dt=F32):
        return nc.declare_dram_parameter(name, list(shape), dt, isOutput=False)

    xsT_d = din("xsT", [F, NPC], BF16)
    gidxlo_d = din("gidxlo", [16, NCHLO * CW], I16)
    gidxhi_d = din("gidxhi", [16, NCHHI * CW], I16)
    sidxlo_d = din("sidxlo", [16, NCHLO * CW], I16)
    sidxhi_d = din("sidxhi", [16, NCHHI * CW], I16)
    dinvb_d = din("dinvb", [P, NBLK])
    batb_d = din("batb", [P, NBLK])
    iota256_d = din("iota256", [P, G])
    invc_rep_d = din("invc_rep", [P, G])
    w0_d = din("w0", [F, F]); wg1_d = din("wg1", [F, F]); wg2_d = din("wg2", [F, F])
    b0c_d = din("b0c", [P, 1]); bg1c_d = din("bg1c", [P, 1])
    bg2rep_d = din("bg2rep", [P, F])
    wh1_d = din("wh1", [F, NHID]); bh1c_d = din("bh1c", [P, 2])
    wh2_d = din("wh2", [NHID, NOUT]); bh2c_d = din("bh2c", [P, 1])
    out_d = nc.declare_dram_parameter("out", [G, NOUT], F32, isOutput=True)

    slices = [nc.dram_tensor(f"slice{L}", [NPC, F], F32) for L in range(3)]
    tabs = [nc.dram_tensor(f"tab{L}", [NPAD, F], F32, addr_space="Shared")
            for L in range(3)]
    accs = [nc.dram_tensor(f"acc{L}", [NPC, F], F32) for L in range(3)]
    pool_in = nc.dram_tensor("pool_in", [P, G], F32)
    pool_out = nc.dram_tensor("pool_out", [P, G], F32, addr_space="Shared")
    groups = [list(range(NCORES))]

    with tile.TileContext(nc) as tc:
        with (
            tc.tile_pool(name="const", bufs=1) as constp,
            tc.tile_pool(name="big", bufs=1) as bigp,
            tc.tile_pool(name="gbuf", bufs=2) as gbufp,
            tc.tile_pool(name="work", bufs=4) as workp,
            tc.tile_pool(name="psy", bufs=2, space="PSUM") as psy,
            tc.tile_pool(name="pst", bufs=2, space="PSUM") as pst,
            tc.tile_pool(name="psp", bufs=1, space="PSUM") as psp,
            tc.tile_pool(name="psh", bufs=2, space="PSUM") as psh,
        ):
            ident = constp.tile([P, P], F32)
            make_identity(nc, ident[:])

            def load(shape, dram, dt=F32, pool=constp):
                t = pool.tile(list(shape), dt, name=f"sb_{dram.name}",
                              tag=f"sb_{dram.name}")
                nc.sync.dma_start(out=t[:], in_=dram[:])
                return t

            w_sb = [load([F, F], d) for d in (w0_d, wg1_d, wg2_d)]
            b0c = load([P, 1], b0c_d)
            bg1c = load([P, 1], bg1c_d)
            bg2rep = load([P, F], bg2rep_d)
            dinvb = load([P, NBLK], dinvb_d)
            batb = load([P, NBLK], batb_d)
            iota256 = load([P, G], iota256_d)
            invc_rep = load([P, G], invc_rep_d)
            wh1 = load([F, NHID], wh1_d)
            bh1c = load([P, 2], bh1c_d)
            bh2c = load([P, 1], bh2c_d)
            wh2 = constp.tile([P, 2 * NOUT], F32)
            for h in range(2):
                nc.sync.dma_start(out=wh2[:, h * NOUT:(h + 1) * NOUT],
                                  in_=wh2_d[h * P:(h + 1) * P, :])
            zero128 = constp.tile([P, P], F32)
            nc.vector.memset(zero128[:], 0.0)

            # idx buffers: load 16 partitions, replicate by doubling to 128
            def load_idx(dram, ncols):
                t = constp.tile([P, ncols], I16, name=f"sb_{dram.name}",
                                tag=f"sb_{dram.name}")
                nc.sync.dma_start(out=t[0:16, :], in_=dram[:])
                for k in (16, 32, 64):
                    nc.sync.dma_start(out=t[k:2 * k, :], in_=t[0:k, :])
                return t

            gidxlo = load_idx(gidxlo_d, NCHLO * CW)
            gidxhi = load_idx(gidxhi_d, NCHHI * CW)
            sidxlo = load_idx(sidxlo_d, NCHLO * CW)
            sidxhi = load_idx(sidxhi_d, NCHHI * CW)

            # bf16 -> f32 cast load of own features (gpsimd DMA casts)
            hT = bigp.tile([F, NPC], F32, name="hT0", tag="hT0")
            nc.gpsimd.dma_start(out=hT[:], in_=xsT_d[:])

            pool_ps = psp.tile([P, G], F32, space="PSUM", tag="pool")

            for L in range(3):
                W = w_sb[L]
                tab, sl, acc = tabs[L], slices[L], accs[L]

                # -- transform own nodes + stage node-major table slice
                with tc.For_i(0, NBLK) as i:
                    ps = psy.tile([P, F], F32, space="PSUM", tag="psy")
                    nc.tensor.matmul(out=ps[:], lhsT=W[:],
                                     rhs=hT[:, ds(i * P, P)],
                                     start=True, stop=True)
                    yT = workp.tile([P, F], F32, tag="yT")
                    nc.vector.tensor_copy(out=yT[:], in_=ps[:])
                    ps2 = pst.tile([P, F], F32, space="PSUM", tag="pst")
                    nc.tensor.transpose(out=ps2[:], in_=yT[:],
                                        identity=ident[:])
                    y = workp.tile([P, F], F32, tag="y")
                    nc.vector.tensor_copy(out=y[:], in_=ps2[:])
                    nc.sync.dma_start(out=sl[ds(i * P, P), :], in_=y[:])

                # -- replicate table; init acc with own y (self/+h term)
                nc.gpsimd.collective_compute(
                    "AllGather", mybir.AluOpType.bypass, replica_groups=groups,
                    ins=[sl[:]], outs=[tab[:]])
                nc.sync.dma_start(out=acc[:], in_=sl[:])

                # -- aggregate all edges: gather src rows, scatter-add to dst
                streams = (("lo", NCHLO, gidxlo, sidxlo, tab[0:NHALF, :]),
                           ("hi", NCHHI, gidxhi, sidxhi, tab[NHALF:NPAD, :]))
                for sname, nch, gx, sx, tab_ap in streams:
                    with tc.For_i(0, nch) as i:
                        buf = gbufp.tile([P, CH], F32, tag="gbuf")
                        nc.gpsimd.dma_gather(
                            out_ap=buf[:].rearrange("p (c f) -> p c f", f=F),
                            in_ap=tab_ap,
                            idxs_ap=gx[:, ds(i * CW, CW)],
                            num_idxs=CH, num_idxs_reg=CH,
                            elem_size=F, single_packet=True,
                            queue_num=(0 if sname == "lo" else 1))
                        nc.gpsimd.dma_scatter_add(
                            acc[:],
                            buf[:].rearrange("p (c f) -> p c f", f=F),
                            sx[:, ds(i * CW, CW)],
                            num_idxs=CH, num_idxs_reg=CH,
                            elem_size=F, single_packet=True,
                            queue_num=2)

                # -- post: per block load, (GCN: *dinv), transpose, bias+relu
                if L < 2:
                    hTn = bigp.tile([F, NPC], F32, name=f"hT{L + 1}",
                                    tag=f"hT{L + 1}")
                    bias = b0c if L == 0 else bg1c
                    with tc.For_i(0, NBLK) as i:
                        a_sb = workp.tile([P, F], F32, tag="a_sb")
                        nc.sync.dma_start(out=a_sb[:],
                                          in_=acc[ds(i * P, P), :])
                        if L == 0:
                            nc.vector.tensor_scalar(
                                out=a_sb[:], in0=a_sb[:],
                                scalar1=dinvb[:, ds(i, 1)], scalar2=None,
                                op0=mybir.AluOpType.mult)
                        ps2 = pst.tile([P, F], F32, space="PSUM", tag="pst")
                        nc.tensor.transpose(out=ps2[:], in_=a_sb[:],
                                            identity=ident[:])
                        nc.scalar.activation(
                            out=hTn[:, ds(i * P, P)], in_=ps2[:],
                            func=mybir.ActivationFunctionType.Relu,
                            bias=bias[:, 0:1])
                    nc.vector.memset(hTn[:, NPN:NPC], 0.0)
                    hT = hTn
                else:
                    # final layer: bias+relu node-major, pool by one-hot
                    # matmul accumulated in PSUM across the loop (dummy
                    # zero-weight matmuls carry the start/stop flags)
                    nc.tensor.matmul(out=pool_ps[:], lhsT=zero128[:],
                                     rhs=iota256[:], start=True, stop=False)
                    with tc.For_i(0, NBLK) as i:
                        a_sb = workp.tile([P, F], F32, tag="a_sb")
                        nc.sync.dma_start(out=a_sb[:],
                                          in_=acc[ds(i * P, P), :])
                        nc.vector.tensor_add(out=a_sb[:], in0=a_sb[:],
                                             in1=bg2rep[:])
                        nc.vector.tensor_scalar_max(a_sb[:], a_sb[:], 0.0)
                        sel = workp.tile([P, G], F32, tag="sel")
                        nc.vector.tensor_scalar(
                            out=sel[:], in0=iota256[:],
                            scalar1=batb[:, ds(i, 1)], scalar2=None,
                            op0=mybir.AluOpType.is_equal)
                        nc.tensor.matmul(out=pool_ps[:], lhsT=a_sb[:],
                                         rhs=sel[:], start=False, stop=False)
                    nc.tensor.matmul(out=pool_ps[:], lhsT=zero128[:],
                                     rhs=iota256[:], start=False, stop=True)

            # ---- pool finish: partial mean -> AllReduce
            mT = workp.tile([P, G], F32, tag="mT")
            nc.vector.tensor_mul(out=mT[:], in0=pool_ps[:], in1=invc_rep[:])
            nc.sync.dma_start(out=pool_in[:], in_=mT[:])
            nc.gpsimd.collective_compute(
                "AllReduce", mybir.AluOpType.add, replica_groups=groups,
                ins=[pool_in[:]], outs=[pool_out[:]])
            mT2 = workp.tile([P, G], F32, tag="mT2")
            nc.sync.dma_start(out=mT2[:], in_=pool_out[:])

            # ---- head (f-major, redundant per core)
            g1T = []
            for h in range(2):
                ps = psh.tile([P, G], F32, space="PSUM", tag="psh")
                nc.tensor.matmul(out=ps[:], lhsT=wh1[:, h * P:(h + 1) * P],
                                 rhs=mT2[:], start=True, stop=True)
                gt = workp.tile([P, G], F32, tag=f"g1T{h}")
                nc.scalar.activation(out=gt[:], in_=ps[:],
                                     func=mybir.ActivationFunctionType.Relu,
                                     bias=bh1c[:, h:h + 1])
                g1T.append(gt)
            o_ps = psh.tile([P, G], F32, space="PSUM", tag="psh")
            for h in range(2):
                nc.tensor.matmul(out=o_ps[:], lhsT=wh2[:, h * NOUT:(h + 1) * NOUT],
                                 rhs=g1T[h][:], start=(h == 0), stop=(h == 1))
            oT = workp.tile([P, G], F32, tag="oT")
            nc.vector.tensor_scalar(out=oT[:], in0=o_ps[:],
                                    scalar1=bh2c[:, 0:1], scalar2=None,
                                    op0=mybir.AluOpType.add)
            for gh in range(2):
                ps = pst.tile([P, NOUT], F32, space="PSUM", tag="pst")
                nc.tensor.transpose(out=ps[:], in_=oT[:, gh * P:(gh + 1) * P],
                                    identity=ident[:])
                o_sb = workp.tile([P, NOUT], F32, tag="o_sb")
                nc.vector.tensor_copy(out=o_sb[:], in_=ps[:])
                nc.sync.dma_start(out=out_d[gh * P:(gh + 1) * P, :], in_=o_sb[:])

    nc.compile()
    return nc


_CACHE = {}


def run(cfg, inputs):
    in_maps, meta = preprocess(cfg, **inputs)
    key = (cfg.N, meta["NCHLO"], meta["NCHHI"])
    if key not in _CACHE:
        _CACHE[key] = build_program(cfg, meta)
    nc = _CACHE[key]
    res = run_bass_kernel_spmd(nc, in_maps, core_ids=list(range(NCORES)))
    return res.results[0]["out"].astype(np.float32)


def kernel(**inputs):
    return run(FULL, inputs)


# revision 13
# speedup vs baseline: 4.6745x; 1.3969x over previous
"""GCN+GIN graph encoder on 8 Trainium2 NeuronCores (Bass/Tile).

Math (reference):
  GCNConv:  h = relu(segsum_dst(norm_e * (x@W0)[src]) + b0),
            norm_e = dinv[src]*dinv[dst] over edges+self-loops,
            dinv = rsqrt(deg incl self-loop)
  GIN x2:   h = relu((h + segsum_dst(h[src])) @ Wg + bg)
  pool:     m = segment_mean(h, batch) -> relu(m@Wh1+bh1)@Wh2+bh2

Distribution: nodes (and their in-edges) sharded contiguously over 8 cores
(6250 nodes/core), weights replicated.

Layer strategy ("transform first, aggregate by DMA"):
  1. y = h @ W on own nodes: per 128-node block, matmul(lhsT=W, rhs=hT_blk)
     -> yT, PE-transpose -> y node-major, DMA to the own table slice.
  2. AllGather own y slice -> replicated table tab[NPAD, F].
  3. acc <- own y slice (one DRAM->DRAM DMA): the self-loop / "+h" term.
  4. All E edges aggregated with dma_gather + dma_scatter_add pairs of
     1024 edges each (src rows gathered from tab, scatter-added into acc
     rows by dst).  dma_scatter_add loses same-row updates issued within
     one call, so edges are dealt so each call hits distinct dst rows
     (sort by dst, call = index mod NCH; NCH >= max degree).  Calls are
     padded to exactly CH with edges that gather slice pad row NPN and
     scatter onto trash row NPN (excluded from all outputs, so pad
     payload values and their races are harmless).
     Gathers split lo/hi on the src table half (int16 index limit).
  5. Post per block: load acc, (GCN: * dinv[dst]), PE-transpose, fused
     bias+relu -> hT block, immediately transformed by the NEXT layer's
     weights and transposed back to stage the next table slice (post and
     transform fused into one hardware loop per layer boundary).
All per-block and per-call loops are For_i hardware loops (dynamic-slice
APs), keeping the NEFF small: per-exec wall overhead here scales with
instruction count and input bytes, not device time.  All small f32
constants ship in one packed [128, C] tensor; gather+scatter indices in
one packed int16 tensor.
Pooling: per-block one-hot(batch) matmul accumulated in PSUM across the
loop (dummy start/stop matmuls bracket it) -> partial mean -> AllReduce;
the 2-layer MLP head runs redundantly per core in f-major; final output
transposed to [G, NOUT].
"""
import sys

sys.path.insert(0, '/opt/trn_rl_repo')

import numpy as np

import concourse.bass as bass
import concourse.bacc as bacc
import concourse.mybir as mybir
import concourse.tile as tile
from concourse.bass import ds
from concourse.bass_utils import run_bass_kernel_spmd
from concourse.masks import make_identity

F32 = mybir.dt.float32
BF16 = mybir.dt.bfloat16
I16 = mybir.dt.int16
P = 128
NCORES = 8
CH = 1024              # edges per gather/scatter call (SWDGE limit)
CW = CH // 16          # idx columns per call (16-partition wrap)
NQ = 4


class Cfg:
    def __init__(self, N, E, G, F, NHID, NOUT, NPN):
        self.N = N
        self.E = E
        self.G = G
        self.F = F
        self.NHID = NHID
        self.NOUT = NOUT
        self.NPN = NPN
        assert NPN * NCORES == N
        self.NPC = ((NPN + P - 1) // P) * P
        self.NBLK = self.NPC // P
        self.NPAD = self.NPC * NCORES
        self.NHALF = self.NPAD // 2
        assert self.NHALF < 32768
        assert NPN < self.NPC          # needs a trash row per slice
        assert G == 2 * P


FULL = Cfg(N=50000, E=800000, G=256, F=128, NHID=256, NOUT=128, NPN=6250)


# ---------------------------------------------------------------- host prep
def preprocess(cfg, x, edge_index, batch, W0, b0, Wg1, bg1, Wg2, bg2,
               Wh1, bh1, Wh2, bh2):
    N, G, F, NPN, NPC, NBLK, NHALF = (cfg.N, cfg.G, cfg.F, cfg.NPN, cfg.NPC,
                                      cfg.NBLK, cfg.NHALF)

    src = np.asarray(edge_index[0], dtype=np.int64)
    dst = np.asarray(edge_index[1], dtype=np.int64)
    batch = np.asarray(batch, dtype=np.int64)

    deg = np.bincount(dst, minlength=N).astype(np.float64) + 1.0  # + self loop
    dinv = (1.0 / np.sqrt(deg)).astype(np.float32)

    c_src = src // NPN
    src_tab = c_src * NPC + (src - c_src * NPN)   # table row of each src

    # pad-edge rows: gather slice pad row, scatter onto trash row NPN
    zrow_lo = NPN                                  # core 0 pad row
    zrow_hi = 4 * NPC + NPN - NHALF                # core 4 pad row, rel half

    c_dst = dst // NPN
    lo_mask = src_tab < NHALF

    nch = {}
    for sname, smask in (("lo", lo_mask), ("hi", ~lo_mask)):
        need = 1
        for c in range(NCORES):
            m = smask & (c_dst == c)
            dcnt = np.bincount(dst[m] - c * NPN, minlength=NPN)
            need = max(need, int(np.ceil(m.sum() / CH)), int(dcnt.max()))
        nch[sname] = need

    def wrap16(arr, n):
        # element i of each CH-chunk -> partition i%16, col base + i//16
        w = arr.reshape(n, CH // 16, 16).transpose(2, 0, 1)
        return w.reshape(16, n * (CH // 16)).astype(np.int16)

    gidx = {"lo": [], "hi": []}
    sidx = {"lo": [], "hi": []}
    for sname, smask, zrow in (("lo", lo_mask, zrow_lo),
                               ("hi", ~lo_mask, zrow_hi)):
        base = 0 if sname == "lo" else NHALF
        n = nch[sname]
        for c in range(NCORES):
            m = smask & (c_dst == c)
            g = (src_tab[m] - base).astype(np.int64)
            s = (dst[m] - c * NPN).astype(np.int64)
            order = np.argsort(s, kind="stable")
            g, s = g[order], s[order]
            ne = len(g)
            call_of = np.arange(ne) % n
            rank = np.arange(ne) // n
            ga = np.full((n, CH), zrow, dtype=np.int64)
            sa = np.full((n, CH), NPN, dtype=np.int64)
            ga[call_of, rank] = g
            sa[call_of, rank] = s
            gidx[sname].append(wrap16(ga.reshape(-1), n))
            sidx[sname].append(wrap16(sa.reshape(-1), n))

    # f-major, dinv-prescaled, padded node features per core (bf16 shipped)
    import ml_dtypes
    xsT = np.zeros((NCORES, F, NPC), dtype=np.float32)
    xk = np.asarray(x, dtype=np.float32) * dinv[:, None]
    for c in range(NCORES):
        xsT[c, :, :NPN] = xk[c * NPN:(c + 1) * NPN].T

    # per-block dst dinv and batch-id columns
    dinv_blk = np.zeros((NCORES, P, NBLK), dtype=np.float32)
    bat_blk = np.full((NCORES, P, NBLK), -1.0, dtype=np.float32)
    for c in range(NCORES):
        dv = np.zeros(NPC, dtype=np.float32)
        dv[:NPN] = dinv[c * NPN:(c + 1) * NPN]
        dinv_blk[c] = dv.reshape(NBLK, P).T
        bb = np.full(NPC, -1.0, dtype=np.float32)
        bb[:NPN] = batch[c * NPN:(c + 1) * NPN].astype(np.float32)
        bat_blk[c] = bb.reshape(NBLK, P).T

    cnt_g = np.bincount(batch, minlength=G).astype(np.float32)
    invc = (1.0 / np.maximum(cnt_g, 1.0)).astype(np.float32)

    # ---- one packed [128, C] f32 constants tensor (column layout below)
    cols = {}

    def put(name, arr):
        arr = np.asarray(arr, np.float32)
        if arr.ndim == 1:
            arr = arr.reshape(P, 1) if arr.shape[0] == P else \
                np.broadcast_to(arr, (P, arr.shape[0])).copy()
        cols[name] = arr
        return arr

    put("w0", W0); put("wg1", Wg1); put("wg2", Wg2)
    put("wh1", Wh1)
    put("wh2a", np.asarray(Wh2, np.float32)[0:P, :])
    put("wh2b", np.asarray(Wh2, np.float32)[P:2 * P, :])
    put("iota256", np.broadcast_to(np.arange(G, dtype=np.float32), (P, G)))
    put("invc_rep", np.broadcast_to(invc, (P, G)))
    put("bg2rep", np.broadcast_to(np.asarray(bg2, np.float32), (P, F)))
    put("b0c", np.asarray(b0, np.float32).reshape(P, 1))
    put("bg1c", np.asarray(bg1, np.float32).reshape(P, 1))
    put("bh1c", np.asarray(bh1, np.float32).reshape(2, P).T)
    put("bh2c", np.asarray(bh2, np.float32).reshape(P, 1))

    layout = {}
    off = 0
    for k, v in cols.items():
        layout[k] = (off, v.shape[1])
        off += v.shape[1]
    # per-core columns appended at the tail
    layout["dinvb"] = (off, NBLK)
    layout["batb"] = (off + NBLK, NBLK)
    CCOLS = off + 2 * NBLK

    packed = np.zeros((NCORES, P, CCOLS), dtype=np.float32)
    for k, v in cols.items():
        o, w = layout[k]
        packed[:, :, o:o + w] = v[None]
    for c in range(NCORES):
        o, w = layout["dinvb"]
        packed[c, :, o:o + w] = dinv_blk[c]
        o, w = layout["batb"]
        packed[c, :, o:o + w] = bat_blk[c]

    # one packed int16 idx tensor: [16, IW] = glo | ghi | slo | shi
    iw = {}
    off = 0
    for nm, d in (("glo", gidx["lo"]), ("ghi", gidx["hi"]),
                  ("slo", sidx["lo"]), ("shi", sidx["hi"])):
        iw[nm] = (off, d[0].shape[1])
        off += d[0].shape[1]
    IW = off
    pidx = np.zeros((NCORES, 16, IW), dtype=np.int16)
    for c in range(NCORES):
        for nm, d in (("glo", gidx["lo"]), ("ghi", gidx["hi"]),
                      ("slo", sidx["lo"]), ("shi", sidx["hi"])):
            o, w = iw[nm]
            pidx[c, :, o:o + w] = d[c]

    in_maps = [dict(xsT=xsT[c].astype(ml_dtypes.bfloat16),
                    consts=packed[c], idx=pidx[c]) for c in range(NCORES)]
    meta = dict(NCHLO=nch["lo"], NCHHI=nch["hi"], layout=layout, iw=iw,
                CCOLS=CCOLS, IW=IW)
    return in_maps, meta


# ---------------------------------------------------------------- program
def build_program(cfg, meta):
    NPC, NBLK, NPAD, NHALF, NPN = (cfg.NPC, cfg.NBLK, cfg.NPAD, cfg.NHALF,
                                   cfg.NPN)
    F, NHID, NOUT, G = cfg.F, cfg.NHID, cfg.NOUT, cfg.G
    NCHLO, NCHHI = meta["NCHLO"], meta["NCHHI"]
    layout, iw, CCOLS, IW = (meta["layout"], meta["iw"], meta["CCOLS"],
                             meta["IW"])

    nc = bacc.Bacc(None, target_bir_lowering=False, debug=True,
                   num_devices=NCORES, num_swdge_queues=NQ)

    xsT_d = nc.declare_dram_parameter("xsT", [F, NPC], BF16, isOutput=False)
    consts_d = nc.declare_dram_parameter("consts", [P, CCOLS], F32,
                                         isOutput=False)
    idx_d = nc.declare_dram_parameter("idx", [16, IW], I16, isOutput=False)
    out_d = nc.declare_dram_parameter("out", [G, NOUT], F32, isOutput=True)

    slices = [nc.dram_tensor(f"slice{L}", [NPC, F], F32) for L in range(3)]
    tabs = [nc.dram_tensor(f"tab{L}", [NPAD, F], F32, addr_space="Shared")
            for L in range(3)]
    accs = [nc.dram_tensor(f"acc{L}", [NPC, F], F32) for L in range(3)]
    pool_in = nc.dram_tensor("pool_in", [P, G], F32)
    pool_out = nc.dram_tensor("pool_out", [P, G], F32, addr_space="Shared")
    groups = [list(range(NCORES))]

    with tile.TileContext(nc) as tc:
        with (
            tc.tile_pool(name="const", bufs=1) as constp,
            tc.tile_pool(name="big", bufs=1) as bigp,
            tc.tile_pool(name="gbuf", bufs=2) as gbufp,
            tc.tile_pool(name="work", bufs=4) as workp,
            tc.tile_pool(name="psy", bufs=2, space="PSUM") as psy,
            tc.tile_pool(name="pst", bufs=2, space="PSUM") as pst,
            tc.tile_pool(name="psp", bufs=1, space="PSUM") as psp,
            tc.tile_pool(name="psh", bufs=2, space="PSUM") as psh,
        ):
            ident = constp.tile([P, P], F32)
            make_identity(nc, ident[:])
            zero128 = constp.tile([P, P], F32)
            nc.vector.memset(zero128[:], 0.0)

            consts = constp.tile([P, CCOLS], F32, name="consts", tag="consts")
            nc.sync.dma_start(out=consts[:], in_=consts_d[:])

            def cc(name, j0=None, j1=None):
                o, w = layout[name]
                if j0 is None:
                    return consts[:, o:o + w]
                return consts[:, o + j0:o + j1]

            def cdyn(name, expr, size):
                o, w = layout[name]
                return consts[:, ds(o + expr, size)]

            w_sb = [cc("w0"), cc("wg1"), cc("wg2")]
            iota256 = cc("iota256")

            idx = constp.tile([P, IW], I16, name="idx", tag="idx")
            nc.sync.dma_start(out=idx[0:16, :], in_=idx_d[:])
            for k in (16, 32, 64):
                nc.sync.dma_start(out=idx[k:2 * k, :], in_=idx[0:k, :])

            def ix(name):
                o, w = iw[name]
                return idx[:, o:o + w], o

            # bf16 -> f32 cast load of own features (gpsimd DMA casts)
            hT = bigp.tile([F, NPC], F32, name="hT0", tag="hT0")
            nc.gpsimd.dma_start(out=hT[:], in_=xsT_d[:])

            pool_ps = psp.tile([P, G], F32, space="PSUM", tag="pool")

            def stage_block(yall, i, W, src_ap):
                """matmul yT=W^T h, transpose, node-major block -> yall col i.

                All writes are engine-synchronous vector ops, so the loop's
                back-edge barrier orders them before the single static DMA
                that follows the loop (dynamic-offset DRAM DMAs inside
                For_i have unreliable cross-boundary dependency tracking).
                """
                ps = psy.tile([P, F], F32, space="PSUM", tag="psy")
                nc.tensor.matmul(out=ps[:], lhsT=W, rhs=src_ap,
                                 start=True, stop=True)
                yT = workp.tile([P, F], F32, tag="yT")
                nc.vector.tensor_copy(out=yT[:], in_=ps[:])
                ps2 = pst.tile([P, F], F32, space="PSUM", tag="pst")
                nc.tensor.transpose(out=ps2[:], in_=yT[:], identity=ident[:])
                nc.vector.tensor_copy(out=yall[:, ds(i * F, F)], in_=ps2[:])

            def flush_slice(yall, sl):
                nc.sync.dma_start(
                    out=sl[:].rearrange("(b p) f -> p b f", p=P),
                    in_=yall[:].rearrange("p (b f) -> p b f", f=F))

            accall_t = bigp.tile([P, NBLK * F], F32, name="accall",
                                 tag="accall")
            yall_t = bigp.tile([P, NBLK * F], F32, name="yall", tag="yall")

            def load_acc(acc):
                nc.sync.dma_start(
                    out=accall_t[:].rearrange("p (b f) -> p b f", f=F),
                    in_=acc[:].rearrange("(b p) f -> p b f", p=P))
                return accall_t

            def emit_agg(L, tab, sl, acc):
                nc.gpsimd.collective_compute(
                    "AllGather", mybir.AluOpType.bypass, replica_groups=groups,
                    ins=[sl[:]], outs=[tab[:]])
                nc.sync.dma_start(out=acc[:], in_=sl[:])
                streams = [("lo", NCHLO, "glo", "slo", tab[0:NHALF, :], 0),
                           ("hi", NCHHI, "ghi", "shi", tab[NHALF:NPAD, :], 1)]
                if NCHLO == NCHHI:
                    with tc.For_i(0, NCHLO) as i:
                        for sname, nch_s, gn, sn, tab_ap, q in streams:
                            gx, go = ix(gn)
                            sx, so = ix(sn)
                            buf = gbufp.tile([P, CH], F32, tag=f"gb{sname}")
                            nc.gpsimd.dma_gather(
                                out_ap=buf[:].rearrange("p (c f) -> p c f",
                                                        f=F),
                                in_ap=tab_ap,
                                idxs_ap=idx[:, ds(go + i * CW, CW)],
                                num_idxs=CH, num_idxs_reg=CH,
                                elem_size=F, single_packet=True, queue_num=q)
                            nc.gpsimd.dma_scatter_add(
                                acc[:],
                                buf[:].rearrange("p (c f) -> p c f", f=F),
                                idx[:, ds(so + i * CW, CW)],
                                num_idxs=CH, num_idxs_reg=CH,
                                elem_size=F, single_packet=True, queue_num=2)
                else:
                    for sname, nch_s, gn, sn, tab_ap, q in streams:
                        gx, go = ix(gn)
                        sx, so = ix(sn)
                        with tc.For_i(0, nch_s) as i:
                            buf = gbufp.tile([P, CH], F32, tag=f"gb{sname}")
                            nc.gpsimd.dma_gather(
                                out_ap=buf[:].rearrange("p (c f) -> p c f",
                                                        f=F),
                                in_ap=tab_ap,
                                idxs_ap=idx[:, ds(go + i * CW, CW)],
                                num_idxs=CH, num_idxs_reg=CH,
                                elem_size=F, single_packet=True, queue_num=q)
                            nc.gpsimd.dma_scatter_add(
                                acc[:],
                                buf[:].rearrange("p (c f) -> p c f", f=F),
                                idx[:, ds(so + i * CW, CW)],
                                num_idxs=CH, num_idxs_reg=CH,
                                elem_size=F, single_packet=True, queue_num=2)

            # ---- layer 0 transform from xsT
            with tc.For_i(0, NBLK) as i:
                stage_block(yall_t, i, w_sb[0], hT[:, ds(i * P, P)])
            flush_slice(yall_t, slices[0])
            emit_agg(0, tabs[0], slices[0], accs[0])

            # ---- fused post(L) + transform(L+1) for L = 0, 1
            for L in (0, 1):
                bname = "b0c" if L == 0 else "bg1c"
                accall = load_acc(accs[L])
                yall = yall_t
                with tc.For_i(0, NBLK) as i:
                    a_sb = workp.tile([P, F], F32, tag="a_sb")
                    if L == 0:
                        nc.vector.tensor_scalar(
                            out=a_sb[:], in0=accall[:, ds(i * F, F)],
                            scalar1=cdyn("dinvb", i, 1), scalar2=None,
                            op0=mybir.AluOpType.mult)
                    else:
                        nc.vector.tensor_copy(out=a_sb[:],
                                              in_=accall[:, ds(i * F, F)])
                    ps2 = pst.tile([P, F], F32, space="PSUM", tag="pst")
                    nc.tensor.transpose(out=ps2[:], in_=a_sb[:],
                                        identity=ident[:])
                    hblk = workp.tile([P, F], F32, tag="hblk")
                    nc.scalar.activation(
                        out=hblk[:], in_=ps2[:],
                        func=mybir.ActivationFunctionType.Relu,
                        bias=cc(bname, 0, 1))
                    stage_block(yall, i, w_sb[L + 1], hblk[:])
                flush_slice(yall, slices[L + 1])
                emit_agg(L + 1, tabs[L + 1], slices[L + 1], accs[L + 1])

            # ---- final layer post: bias+relu node-major, one-hot pool
            bg2rep = cc("bg2rep")
            accall2 = load_acc(accs[2])
            nc.tensor.matmul(out=pool_ps[:], lhsT=zero128[:],
                             rhs=iota256, start=True, stop=False)
            with tc.For_i(0, NBLK) as i:
                a_sb = workp.tile([P, F], F32, tag="a_sb")
                nc.vector.tensor_add(out=a_sb[:], in0=accall2[:, ds(i * F, F)],
                                     in1=bg2rep)
                nc.vector.tensor_scalar_max(a_sb[:], a_sb[:], 0.0)
                sel = workp.tile([P, G], F32, tag="sel")
                nc.vector.tensor_scalar(
                    out=sel[:], in0=iota256,
                    scalar1=cdyn("batb", i, 1), scalar2=None,
                    op0=mybir.AluOpType.is_equal)
                nc.tensor.matmul(out=pool_ps[:], lhsT=a_sb[:], rhs=sel[:],
                                 start=False, stop=False)
            nc.tensor.matmul(out=pool_ps[:], lhsT=zero128[:],
                             rhs=iota256, start=False, stop=True)

            # ---- pool finish: partial mean -> AllReduce
            mT = workp.tile([P, G], F32, tag="mT")
            nc.vector.tensor_mul(out=mT[:], in0=pool_ps[:], in1=cc("invc_rep"))
            nc.sync.dma_start(out=pool_in[:], in_=mT[:])
            nc.gpsimd.collective_compute(
                "AllReduce", mybir.AluOpType.add, replica_groups=groups,
                ins=[pool_in[:]], outs=[pool_out[:]])
            mT2 = workp.tile([P, G], F32, tag="mT2")
            nc.sync.dma_start(out=mT2[:], in_=pool_out[:])

            # ---- head (f-major, redundant per core)
            g1T = []
            for h in range(2):
                ps = psh.tile([P, G], F32, space="PSUM", tag="psh")
                nc.tensor.matmul(out=ps[:], lhsT=cc("wh1", h * P, (h + 1) * P),
                                 rhs=mT2[:], start=True, stop=True)
                gt = workp.tile([P, G], F32, tag=f"g1T{h}")
                nc.scalar.activation(out=gt[:], in_=ps[:],
                                     func=mybir.ActivationFunctionType.Relu,
                                     bias=cc("bh1c", h, h + 1))
                g1T.append(gt)
            o_ps = psh.tile([P, G], F32, space="PSUM", tag="psh")
            for h, wname in enumerate(("wh2a", "wh2b")):
                nc.tensor.matmul(out=o_ps[:], lhsT=cc(wname),
                                 rhs=g1T[h][:], start=(h == 0), stop=(h == 1))
            oT = workp.tile([P, G], F32, tag="oT")
            nc.vector.tensor_scalar(out=oT[:], in0=o_ps[:],
                                    scalar1=cc("bh2c", 0, 1), scalar2=None,
                                    op0=mybir.AluOpType.add)
            for gh in range(2):
                ps = pst.tile([P, NOUT], F32, space="PSUM", tag="pst")
                nc.tensor.transpose(out=ps[:], in_=oT[:, gh * P:(gh + 1) * P],
                                    identity=ident[:])
                o_sb = workp.tile([P, NOUT], F32, tag="o_sb")
                nc.vector.tensor_copy(out=o_sb[:], in_=ps[:])
                nc.sync.dma_start(out=out_d[gh * P:(gh + 1) * P, :], in_=o_sb[:])

    nc.compile()
    return nc


_CACHE = {}


def run(cfg, inputs):
    in_maps, meta = preprocess(cfg, **inputs)
    key = (cfg.N, meta["NCHLO"], meta["NCHHI"])
    if key not in _CACHE:
        _CACHE[key] = build_program(cfg, meta)
    nc = _CACHE[key]
    res = run_bass_kernel_spmd(nc, in_maps, core_ids=list(range(NCORES)))
    return res.results[0]["out"].astype(np.float32)


def kernel(**inputs):
    return run(FULL, inputs)


# revision 15
# speedup vs baseline: 5.1718x; 1.1064x over previous
"""GCN+GIN graph encoder on 8 Trainium2 NeuronCores (Bass/Tile).

Math (reference):
  GCNConv:  h = relu(segsum_dst(norm_e * (x@W0)[src]) + b0),
            norm_e = dinv[src]*dinv[dst] over edges+self-loops,
            dinv = rsqrt(deg incl self-loop)
  GIN x2:   h = relu((h + segsum_dst(h[src])) @ Wg + bg)
  pool:     m = segment_mean(h, batch) -> relu(m@Wh1+bh1)@Wh2+bh2

Distribution: nodes (and their in-edges) sharded contiguously over 8 cores
(6250 nodes/core), weights replicated.

Layer strategy ("transform first, aggregate by DMA"):
  1. y = h @ W on own nodes: per 128-node block, matmul(lhsT=W, rhs=hT_blk)
     -> yT, PE-transpose -> y node-major, DMA to the own table slice.
  2. AllGather own y slice -> replicated table tab[NPAD, F].
  3. acc <- own y slice (one DRAM->DRAM DMA): the self-loop / "+h" term.
  4. All E edges aggregated with dma_gather + dma_scatter_add pairs of
     1024 edges each (src rows gathered from tab, scatter-added into acc
     rows by dst).  dma_scatter_add loses same-row updates issued within
     one call, so edges are dealt so each call hits distinct dst rows
     (sort by dst, call = index mod NCH; NCH >= max degree).  Calls are
     padded to exactly CH with edges that gather slice pad row NPN and
     scatter onto trash row NPN (excluded from all outputs, so pad
     payload values and their races are harmless).
     Gathers split lo/hi on the src table half (int16 index limit).
  5. Post per block: load acc, (GCN: * dinv[dst]), PE-transpose, fused
     bias+relu -> hT block, immediately transformed by the NEXT layer's
     weights and transposed back to stage the next table slice (post and
     transform fused into one hardware loop per layer boundary).
All per-block and per-call loops are For_i hardware loops (dynamic-slice
APs), keeping the NEFF small: per-exec wall overhead here scales with
instruction count and input bytes, not device time.  All small f32
constants ship in one packed [128, C] tensor; gather+scatter indices in
one packed int16 tensor.
Pooling: per-block one-hot(batch) matmul accumulated in PSUM across the
loop (dummy start/stop matmuls bracket it) -> partial mean -> AllReduce;
the 2-layer MLP head runs redundantly per core in f-major; final output
transposed to [G, NOUT].
"""
import sys

sys.path.insert(0, '/opt/trn_rl_repo')

import numpy as np

import concourse.bass as bass
import concourse.bacc as bacc
import concourse.mybir as mybir
import concourse.tile as tile
from concourse.bass import ds
from concourse.bass_utils import run_bass_kernel_spmd
from concourse.masks import make_identity

F32 = mybir.dt.float32
BF16 = mybir.dt.bfloat16
I16 = mybir.dt.int16
P = 128
NCORES = 8
CH = 1024              # edges per gather/scatter call (SWDGE limit)
CW = CH // 16          # idx columns per call (16-partition wrap)
NQ = 4


class Cfg:
    def __init__(self, N, E, G, F, NHID, NOUT, NPN):
        self.N = N
        self.E = E
        self.G = G
        self.F = F
        self.NHID = NHID
        self.NOUT = NOUT
        self.NPN = NPN
        assert NPN * NCORES == N
        self.NPC = ((NPN + P - 1) // P) * P
        self.NBLK = self.NPC // P
        self.NPAD = self.NPC * NCORES
        self.NHALF = self.NPAD // 2
        assert self.NHALF < 32768
        assert NPN < self.NPC          # needs a trash row per slice
        assert G == 2 * P


FULL = Cfg(N=50000, E=800000, G=256, F=128, NHID=256, NOUT=128, NPN=6250)


# ---------------------------------------------------------------- host prep
def preprocess(cfg, x, edge_index, batch, W0, b0, Wg1, bg1, Wg2, bg2,
               Wh1, bh1, Wh2, bh2):
    N, G, F, NPN, NPC, NBLK, NHALF = (cfg.N, cfg.G, cfg.F, cfg.NPN, cfg.NPC,
                                      cfg.NBLK, cfg.NHALF)

    src = np.asarray(edge_index[0], dtype=np.int64)
    dst = np.asarray(edge_index[1], dtype=np.int64)
    batch = np.asarray(batch, dtype=np.int64)

    deg = np.bincount(dst, minlength=N).astype(np.float64) + 1.0  # + self loop
    dinv = (1.0 / np.sqrt(deg)).astype(np.float32)

    c_src = src // NPN
    src_tab = c_src * NPC + (src - c_src * NPN)   # table row of each src

    # pad-edge rows: gather slice pad row, scatter onto trash row NPN
    zrow_lo = NPN                                  # core 0 pad row
    zrow_hi = 4 * NPC + NPN - NHALF                # core 4 pad row, rel half

    c_dst = dst // NPN
    lo_mask = src_tab < NHALF

    nch = {}
    for sname, smask in (("lo", lo_mask), ("hi", ~lo_mask)):
        need = 1
        for c in range(NCORES):
            m = smask & (c_dst == c)
            dcnt = np.bincount(dst[m] - c * NPN, minlength=NPN)
            need = max(need, int(np.ceil(m.sum() / CH)), int(dcnt.max()))
        nch[sname] = need

    def wrap16(arr, n):
        # element i of each CH-chunk -> partition i%16, col base + i//16
        w = arr.reshape(n, CH // 16, 16).transpose(2, 0, 1)
        return w.reshape(16, n * (CH // 16)).astype(np.int16)

    gidx = {"lo": [], "hi": []}
    sidx = {"lo": [], "hi": []}
    for sname, smask, zrow in (("lo", lo_mask, zrow_lo),
                               ("hi", ~lo_mask, zrow_hi)):
        base = 0 if sname == "lo" else NHALF
        n = nch[sname]
        for c in range(NCORES):
            m = smask & (c_dst == c)
            g = (src_tab[m] - base).astype(np.int64)
            s = (dst[m] - c * NPN).astype(np.int64)
            order = np.argsort(s, kind="stable")
            g, s = g[order], s[order]
            ne = len(g)
            call_of = np.arange(ne) % n
            rank = np.arange(ne) // n
            ga = np.full((n, CH), zrow, dtype=np.int64)
            sa = np.full((n, CH), NPN, dtype=np.int64)
            ga[call_of, rank] = g
            sa[call_of, rank] = s
            gidx[sname].append(wrap16(ga.reshape(-1), n))
            sidx[sname].append(wrap16(sa.reshape(-1), n))

    # f-major, dinv-prescaled, padded node features per core (bf16 shipped)
    import ml_dtypes
    xsT = np.zeros((NCORES, F, NPC), dtype=np.float32)
    xk = np.asarray(x, dtype=np.float32) * dinv[:, None]
    for c in range(NCORES):
        xsT[c, :, :NPN] = xk[c * NPN:(c + 1) * NPN].T

    # per-block dst dinv and batch-id columns
    dinv_blk = np.zeros((NCORES, P, NBLK), dtype=np.float32)
    bat_blk = np.full((NCORES, P, NBLK), -1.0, dtype=np.float32)
    for c in range(NCORES):
        dv = np.zeros(NPC, dtype=np.float32)
        dv[:NPN] = dinv[c * NPN:(c + 1) * NPN]
        dinv_blk[c] = dv.reshape(NBLK, P).T
        bb = np.full(NPC, -1.0, dtype=np.float32)
        bb[:NPN] = batch[c * NPN:(c + 1) * NPN].astype(np.float32)
        bat_blk[c] = bb.reshape(NBLK, P).T

    cnt_g = np.bincount(batch, minlength=G).astype(np.float32)
    invc = (1.0 / np.maximum(cnt_g, 1.0)).astype(np.float32)

    # ---- one packed [128, C] f32 constants tensor (column layout below)
    cols = {}

    def put(name, arr):
        arr = np.asarray(arr, np.float32)
        if arr.ndim == 1:
            arr = arr.reshape(P, 1) if arr.shape[0] == P else \
                np.broadcast_to(arr, (P, arr.shape[0])).copy()
        cols[name] = arr
        return arr

    put("w0", W0); put("wg1", Wg1); put("wg2", Wg2)
    put("wh1", Wh1)
    put("wh2a", np.asarray(Wh2, np.float32)[0:P, :])
    put("wh2b", np.asarray(Wh2, np.float32)[P:2 * P, :])
    put("iota256", np.broadcast_to(np.arange(G, dtype=np.float32), (P, G)))
    put("invc_rep", np.broadcast_to(invc, (P, G)))
    put("bg2rep", np.broadcast_to(np.asarray(bg2, np.float32), (P, F)))
    put("b0c", np.asarray(b0, np.float32).reshape(P, 1))
    put("bg1c", np.asarray(bg1, np.float32).reshape(P, 1))
    put("bh1c", np.asarray(bh1, np.float32).reshape(2, P).T)
    put("bh2c", np.asarray(bh2, np.float32).reshape(P, 1))

    layout = {}
    off = 0
    for k, v in cols.items():
        layout[k] = (off, v.shape[1])
        off += v.shape[1]
    # per-core columns appended at the tail
    layout["dinvb"] = (off, NBLK)
    layout["batb"] = (off + NBLK, NBLK)
    CCOLS = off + 2 * NBLK

    packed = np.zeros((NCORES, P, CCOLS), dtype=np.float32)
    for k, v in cols.items():
        o, w = layout[k]
        packed[:, :, o:o + w] = v[None]
    for c in range(NCORES):
        o, w = layout["dinvb"]
        packed[c, :, o:o + w] = dinv_blk[c]
        o, w = layout["batb"]
        packed[c, :, o:o + w] = bat_blk[c]

    # one packed int16 idx tensor: [16, IW] = glo | ghi | slo | shi
    iw = {}
    off = 0
    for nm, d in (("glo", gidx["lo"]), ("ghi", gidx["hi"]),
                  ("slo", sidx["lo"]), ("shi", sidx["hi"])):
        iw[nm] = (off, d[0].shape[1])
        off += d[0].shape[1]
    IW = off
    pidx = np.zeros((NCORES, 16, IW), dtype=np.int16)
    for c in range(NCORES):
        for nm, d in (("glo", gidx["lo"]), ("ghi", gidx["hi"]),
                      ("slo", sidx["lo"]), ("shi", sidx["hi"])):
            o, w = iw[nm]
            pidx[c, :, o:o + w] = d[c]

    in_maps = [dict(xsT=xsT[c].astype(ml_dtypes.bfloat16),
                    consts=packed[c], idx=pidx[c]) for c in range(NCORES)]
    meta = dict(NCHLO=nch["lo"], NCHHI=nch["hi"], layout=layout, iw=iw,
                CCOLS=CCOLS, IW=IW)
    return in_maps, meta


# ---------------------------------------------------------------- program
def build_program(cfg, meta):
    NPC, NBLK, NPAD, NHALF, NPN = (cfg.NPC, cfg.NBLK, cfg.NPAD, cfg.NHALF,
                                   cfg.NPN)
    F, NHID, NOUT, G = cfg.F, cfg.NHID, cfg.NOUT, cfg.G
    NCHLO, NCHHI = meta["NCHLO"], meta["NCHHI"]
    layout, iw, CCOLS, IW = (meta["layout"], meta["iw"], meta["CCOLS"],
                             meta["IW"])

    nc = bacc.Bacc(None, target_bir_lowering=False, debug=True,
                   num_devices=NCORES, num_swdge_queues=NQ)

    xsT_d = nc.declare_dram_parameter("xsT", [F, NPC], BF16, isOutput=False)
    consts_d = nc.declare_dram_parameter("consts", [P, CCOLS], F32,
                                         isOutput=False)
    idx_d = nc.declare_dram_parameter("idx", [16, IW], I16, isOutput=False)
    out_d = nc.declare_dram_parameter("out", [G, NOUT], F32, isOutput=True)

    slices = [nc.dram_tensor(f"slice{L}", [NPC, F], F32) for L in range(3)]
    tabs = [nc.dram_tensor(f"tab{L}", [NPAD, F], F32, addr_space="Shared")
            for L in range(3)]
    accs = [nc.dram_tensor(f"acc{L}", [NPC, F], F32) for L in range(3)]
    pool_in = nc.dram_tensor("pool_in", [P, G], F32)
    pool_out = nc.dram_tensor("pool_out", [P, G], F32, addr_space="Shared")
    groups = [list(range(NCORES))]

    with tile.TileContext(nc) as tc:
        with (
            tc.tile_pool(name="const", bufs=1) as constp,
            tc.tile_pool(name="big", bufs=1) as bigp,
            tc.tile_pool(name="gbuf", bufs=2) as gbufp,
            tc.tile_pool(name="work", bufs=4) as workp,
            tc.tile_pool(name="psy", bufs=2, space="PSUM") as psy,
            tc.tile_pool(name="pst", bufs=2, space="PSUM") as pst,
            tc.tile_pool(name="psp", bufs=1, space="PSUM") as psp,
            tc.tile_pool(name="psh", bufs=2, space="PSUM") as psh,
        ):
            ident = constp.tile([P, P], F32)
            make_identity(nc, ident[:])
            zero128 = constp.tile([P, P], F32)
            nc.vector.memset(zero128[:], 0.0)

            consts = constp.tile([P, CCOLS], F32, name="consts", tag="consts")
            nc.sync.dma_start(out=consts[:], in_=consts_d[:])

            def cc(name, j0=None, j1=None):
                o, w = layout[name]
                if j0 is None:
                    return consts[:, o:o + w]
                return consts[:, o + j0:o + j1]

            def cdyn(name, expr, size):
                o, w = layout[name]
                return consts[:, ds(o + expr, size)]

            w_sb = [cc("w0"), cc("wg1"), cc("wg2")]
            iota256 = cc("iota256")

            idx = constp.tile([P, IW], I16, name="idx", tag="idx")
            nc.sync.dma_start(out=idx[0:16, :], in_=idx_d[:])
            for k in (16, 32, 64):
                nc.sync.dma_start(out=idx[k:2 * k, :], in_=idx[0:k, :])

            def ix(name):
                o, w = iw[name]
                return idx[:, o:o + w], o

            # bf16 -> f32 cast load of own features (gpsimd DMA casts)
            hT = bigp.tile([F, NPC], F32, name="hT0", tag="hT0")
            nc.gpsimd.dma_start(out=hT[:], in_=xsT_d[:])

            pool_ps = psp.tile([P, G], F32, space="PSUM", tag="pool")

            def stage_block(yall, i, W, src_ap):
                """matmul yT=W^T h, transpose, node-major block -> yall col i.

                All writes are engine-synchronous vector ops, so the loop's
                back-edge barrier orders them before the single static DMA
                that follows the loop (dynamic-offset DRAM DMAs inside
                For_i have unreliable cross-boundary dependency tracking).
                """
                ps = psy.tile([P, F], F32, space="PSUM", tag="psy")
                nc.tensor.matmul(out=ps[:], lhsT=W, rhs=src_ap,
                                 start=True, stop=True)
                yT = workp.tile([P, F], F32, tag="yT")
                nc.vector.tensor_copy(out=yT[:], in_=ps[:])
                ps2 = pst.tile([P, F], F32, space="PSUM", tag="pst")
                nc.tensor.transpose(out=ps2[:], in_=yT[:], identity=ident[:])
                nc.vector.tensor_copy(out=yall[:, ds(i * F, F)], in_=ps2[:])

            def flush_slice(yall, sl):
                nc.sync.dma_start(
                    out=sl[:].rearrange("(b p) f -> p b f", p=P),
                    in_=yall[:].rearrange("p (b f) -> p b f", f=F))

            accall_t = bigp.tile([P, NBLK * F], F32, name="accall",
                                 tag="accall")
            yall_t = bigp.tile([P, NBLK * F], F32, name="yall", tag="yall")

            def load_acc(acc):
                nc.sync.dma_start(
                    out=accall_t[:].rearrange("p (b f) -> p b f", f=F),
                    in_=acc[:].rearrange("(b p) f -> p b f", p=P))
                return accall_t

            def emit_agg(L, tab, sl, acc):
                nc.gpsimd.collective_compute(
                    "AllGather", mybir.AluOpType.bypass, replica_groups=groups,
                    ins=[sl[:]], outs=[tab[:]])
                nc.sync.dma_start(out=acc[:], in_=sl[:])
                streams = [("lo", NCHLO, "glo", "slo", tab[0:NHALF, :], 0),
                           ("hi", NCHHI, "ghi", "shi", tab[NHALF:NPAD, :], 1)]
                if NCHLO == NCHHI:
                    with tc.For_i(0, NCHLO) as i:
                        for sname, nch_s, gn, sn, tab_ap, q in streams:
                            gx, go = ix(gn)
                            sx, so = ix(sn)
                            buf = gbufp.tile([P, CH], F32, tag=f"gb{sname}")
                            nc.gpsimd.dma_gather(
                                out_ap=buf[:].rearrange("p (c f) -> p c f",
                                                        f=F),
                                in_ap=tab_ap,
                                idxs_ap=idx[:, ds(go + i * CW, CW)],
                                num_idxs=CH, num_idxs_reg=CH,
                                elem_size=F, single_packet=True, queue_num=q)
                            nc.gpsimd.dma_scatter_add(
                                acc[:],
                                buf[:].rearrange("p (c f) -> p c f", f=F),
                                idx[:, ds(so + i * CW, CW)],
                                num_idxs=CH, num_idxs_reg=CH,
                                elem_size=F, single_packet=True, queue_num=2)
                else:
                    for sname, nch_s, gn, sn, tab_ap, q in streams:
                        gx, go = ix(gn)
                        sx, so = ix(sn)
                        with tc.For_i(0, nch_s) as i:
                            buf = gbufp.tile([P, CH], F32, tag=f"gb{sname}")
                            nc.gpsimd.dma_gather(
                                out_ap=buf[:].rearrange("p (c f) -> p c f",
                                                        f=F),
                                in_ap=tab_ap,
                                idxs_ap=idx[:, ds(go + i * CW, CW)],
                                num_idxs=CH, num_idxs_reg=CH,
                                elem_size=F, single_packet=True, queue_num=q)
                            nc.gpsimd.dma_scatter_add(
                                acc[:],
                                buf[:].rearrange("p (c f) -> p c f", f=F),
                                idx[:, ds(so + i * CW, CW)],
                                num_idxs=CH, num_idxs_reg=CH,
                                elem_size=F, single_packet=True, queue_num=2)

            # ---- layer 0 transform from xsT
            with tc.For_i(0, NBLK) as i:
                stage_block(yall_t, i, w_sb[0], hT[:, ds(i * P, P)])
            flush_slice(yall_t, slices[0])
            emit_agg(0, tabs[0], slices[0], accs[0])

            # ---- fused post(L) + transform(L+1) for L = 0, 1
            for L in (0, 1):
                bname = "b0c" if L == 0 else "bg1c"
                accall = load_acc(accs[L])
                yall = yall_t
                with tc.For_i(0, NBLK) as i:
                    a_sb = workp.tile([P, F], F32, tag="a_sb")
                    if L == 0:
                        nc.vector.tensor_scalar(
                            out=a_sb[:], in0=accall[:, ds(i * F, F)],
                            scalar1=cdyn("dinvb", i, 1), scalar2=None,
                            op0=mybir.AluOpType.mult)
                    else:
                        nc.vector.tensor_copy(out=a_sb[:],
                                              in_=accall[:, ds(i * F, F)])
                    ps2 = pst.tile([P, F], F32, space="PSUM", tag="pst")
                    nc.tensor.transpose(out=ps2[:], in_=a_sb[:],
                                        identity=ident[:])
                    hblk = workp.tile([P, F], F32, tag="hblk")
                    nc.scalar.activation(
                        out=hblk[:], in_=ps2[:],
                        func=mybir.ActivationFunctionType.Relu,
                        bias=cc(bname, 0, 1))
                    stage_block(yall, i, w_sb[L + 1], hblk[:])
                flush_slice(yall, slices[L + 1])
                emit_agg(L + 1, tabs[L + 1], slices[L + 1], accs[L + 1])

            # ---- final layer post: bias+relu node-major, one-hot pool
            bg2rep = cc("bg2rep")
            accall2 = load_acc(accs[2])
            nc.tensor.matmul(out=pool_ps[:], lhsT=zero128[:],
                             rhs=iota256, start=True, stop=False)
            with tc.For_i(0, NBLK) as i:
                a_sb = workp.tile([P, F], F32, tag="a_sb")
                nc.vector.tensor_add(out=a_sb[:], in0=accall2[:, ds(i * F, F)],
                                     in1=bg2rep)
                nc.vector.tensor_scalar_max(a_sb[:], a_sb[:], 0.0)
                sel = workp.tile([P, G], F32, tag="sel")
                nc.vector.tensor_scalar(
                    out=sel[:], in0=iota256,
                    scalar1=cdyn("batb", i, 1), scalar2=None,
                    op0=mybir.AluOpType.is_equal)
                nc.tensor.matmul(out=pool_ps[:], lhsT=a_sb[:], rhs=sel[:],
                                 start=False, stop=False)
            nc.tensor.matmul(out=pool_ps[:], lhsT=zero128[:],
                             rhs=iota256, start=False, stop=True)

            # ---- pool finish: partial mean -> AllReduce
            mT = workp.tile([P, G], F32, tag="mT")
            nc.vector.tensor_mul(out=mT[:], in0=pool_ps[:], in1=cc("invc_rep"))
            nc.sync.dma_start(out=pool_in[:], in_=mT[:])
            nc.gpsimd.collective_compute(
                "AllReduce", mybir.AluOpType.add, replica_groups=groups,
                ins=[pool_in[:]], outs=[pool_out[:]])
            mT2 = workp.tile([P, G], F32, tag="mT2")
            nc.sync.dma_start(out=mT2[:], in_=pool_out[:])

            # ---- head (f-major, redundant per core)
            g1T = []
            for h in range(2):
                ps = psh.tile([P, G], F32, space="PSUM", tag="psh")
                nc.tensor.matmul(out=ps[:], lhsT=cc("wh1", h * P, (h + 1) * P),
                                 rhs=mT2[:], start=True, stop=True)
                gt = workp.tile([P, G], F32, tag=f"g1T{h}")
                nc.scalar.activation(out=gt[:], in_=ps[:],
                                     func=mybir.ActivationFunctionType.Relu,
                                     bias=cc("bh1c", h, h + 1))
                g1T.append(gt)
            o_ps = psh.tile([P, G], F32, space="PSUM", tag="psh")
            for h, wname in enumerate(("wh2a", "wh2b")):
                nc.tensor.matmul(out=o_ps[:], lhsT=cc(wname),
                                 rhs=g1T[h][:], start=(h == 0), stop=(h == 1))
            oT = workp.tile([P, G], F32, tag="oT")
            nc.vector.tensor_scalar(out=oT[:], in0=o_ps[:],
                                    scalar1=cc("bh2c", 0, 1), scalar2=None,
                                    op0=mybir.AluOpType.add)
            for gh in range(2):
                ps = pst.tile([P, NOUT], F32, space="PSUM", tag="pst")
                nc.tensor.transpose(out=ps[:], in_=oT[:, gh * P:(gh + 1) * P],
                                    identity=ident[:])
                o_sb = workp.tile([P, NOUT], F32, tag="o_sb")
                nc.vector.tensor_copy(out=o_sb[:], in_=ps[:])
                nc.sync.dma_start(out=out_d[gh * P:(gh + 1) * P, :], in_=o_sb[:])

    nc.compile()
    return nc


_CACHE = {}


def run(cfg, inputs):
    in_maps, meta = preprocess(cfg, **inputs)
    key = (cfg.N, meta["NCHLO"], meta["NCHHI"])
    if key not in _CACHE:
        _CACHE[key] = build_program(cfg, meta)
    nc = _CACHE[key]
    res = run_bass_kernel_spmd(nc, in_maps, core_ids=list(range(NCORES)))
    return res.results[0]["out"].astype(np.float32)


def kernel(**inputs):
    return run(FULL, inputs)
